# revision 1
# baseline (speedup 1.0000x reference)
"""Trainium2 Bass kernel for nn_ConvBin: 1x1 conv (512->32) + sign(tanh(.)).

The 1x1 conv over NHWC [32,64,64,512] with HWIO [1,1,512,32] is a plain
matmul: out[131072, 32] = x[131072, 512] @ W[512, 32], followed by
sign(tanh(y)) == sign(y) elementwise (tanh is sign-preserving, incl. 0).

Strategy (data-parallel over batch, 8 cores, 4 images each = 16384 rows,
processed as 128 chunks of 128 rows):
    - DMA x in 2 MB loads (8 chunks) naturally: rows on partitions,
      2 KB contiguous per partition per chunk; few, large dma_starts to
      amortize the ~625 ns HWDGE fixed cost per DMA
    - PE-transpose each chunk's four [128,128] k-tiles (fp32 transpose
      mode, 2 cyc/row) into 2-chunk PSUM tiles
    - copy PSUM->SBUF with one ScalarE + one VectorE op per 2 chunks
      (split so both engines stay under the DMA roofline)
    - per chunk, 4 accumulating fp32 matmuls: lhsT = xT tile (stationary,
      k on partitions), rhs = W k-tile [128, 32] (moving; the fp32
      4-cyc/row penalty only hits these 32 columns); 8 chunks accumulate
      into one PSUM bank
    - one Sign per 8 chunks via ScalarE activation LUT, straight from PSUM
    - one 256 KB output DMA per 16 chunks

W is pre-packed on host to [128, 4*32] (k-tiles side by side); identity for
the PE transpose is host-provided. Both are tiny (64 KB).

Verified bit-exact against the jax fp32 reference (0/4194304 mismatches).
HW-calibrated cost model (TimelineSim) estimate: ~123 us end-to-end per
core (DMA-bound; input streaming floor is ~93 us at ~360 GB/s per core).
"""

import numpy as np

import concourse.tile as tile
from concourse import bacc, mybir
from concourse._compat import get_trn_type
from concourse.bass_utils import run_bass_kernel_spmd

N_CORES = 8
B, H, W_DIM, C_IN, C_OUT = 32, 64, 64, 512, 32
ROWS = (B // N_CORES) * H * W_DIM  # 16384 rows per core
KC = C_IN // 128  # 4 k-tiles
N_CHUNKS = ROWS // 128  # 128 chunks of 128 rows

_NC = {}


def _build(reps=1):
    nc = bacc.Bacc(
        get_trn_type() or "TRN2",
        target_bir_lowering=False,
        debug=False,
        num_devices=N_CORES,
    )
    x_in = nc.dram_tensor("x", [ROWS, C_IN], mybir.dt.float32, kind="ExternalInput")
    w_in = nc.dram_tensor("w", [128, KC * C_OUT], mybir.dt.float32, kind="ExternalInput")
    id_in = nc.dram_tensor("ident", [128, 128], mybir.dt.float32, kind="ExternalInput")
    y_out = nc.dram_tensor("y", [ROWS, C_OUT], mybir.dt.float32, kind="ExternalOutput")

    # Loop structure: 8 groups x 16 chunks (of 128 rows each).
    # - input DMA: 4 chunks (512 rows, 1 MB) per dma_start, to amortize the
    #   ~625ns HWDGE fixed cost (the v1 bottleneck per the cost model)
    # - output DMA: 16 chunks (one group, 256 KB) per dma_start
    GROUPS = 8
    CHUNKS_PER_GROUP = 16
    CHUNKS_PER_LOAD = 8
    # PSUM->SBUF xT copy split over a 2-chunk (1024-col) PSUM tile:
    # ScalarE (1.2 GHz) takes [0:ACT_COLS], VectorE (0.96 GHz) the rest.
    ACT_COLS = 560

    with tile.TileContext(nc) as tc:
        with (
            tc.tile_pool(name="consts", bufs=1) as consts,
            tc.tile_pool(name="xin", bufs=3) as xin_pool,
            tc.tile_pool(name="xt", bufs=5) as xt_pool,
            tc.tile_pool(name="psum_t", bufs=2, space="PSUM") as psum_t_pool,
            tc.tile_pool(name="psum_o", bufs=2, space="PSUM") as psum_o_pool,
            tc.tile_pool(name="osb", bufs=2) as out_pool,
        ):
            w_sb = consts.tile([128, KC * C_OUT], mybir.dt.float32)
            nc.sync.dma_start(out=w_sb[:], in_=w_in[:])
            id_sb = consts.tile([128, 128], mybir.dt.float32)
            nc.sync.dma_start(out=id_sb[:], in_=id_in[:])

            rows_per_load = CHUNKS_PER_LOAD * 128
            rows_per_group = CHUNKS_PER_GROUP * 128
            for g in range(GROUPS * reps):
                g = g % GROUPS
                o_sb = out_pool.tile([128, CHUNKS_PER_GROUP * C_OUT], mybir.dt.float32)
                for s in range(CHUNKS_PER_GROUP // CHUNKS_PER_LOAD):
                    x_sb = xin_pool.tile([128, CHUNKS_PER_LOAD * C_IN], mybir.dt.float32)
                    r0 = g * rows_per_group + s * rows_per_load
                    nc.sync.dma_start(
                        out=x_sb[:].rearrange("p (rc k) -> p rc k", k=C_IN),
                        in_=x_in[r0:r0 + rows_per_load, :].rearrange(
                            "(rc p) k -> p rc k", p=128
                        ),
                    )
                    # Transpose 2 chunks into one 2-bank PSUM tile, then copy
                    # out with one ScalarE + one VectorE op per pair.
                    xts = []
                    for h in range(CHUNKS_PER_LOAD // 2):
                        pt2 = psum_t_pool.tile([128, 2 * C_IN], mybir.dt.float32)
                        for rc2 in range(2):
                            xoff = (2 * h + rc2) * C_IN
                            for k in range(KC):
                                nc.tensor.transpose(
                                    pt2[:, rc2 * C_IN + k * 128:rc2 * C_IN + (k + 1) * 128],
                                    x_sb[:, xoff + k * 128:xoff + (k + 1) * 128],
                                    id_sb[:],
                                )
                        xt2 = xt_pool.tile([128, 2 * C_IN], mybir.dt.float32)
                        nc.scalar.activation(
                            xt2[:, 0:ACT_COLS],
                            pt2[:, 0:ACT_COLS],
                            mybir.ActivationFunctionType.Copy,
                        )
                        nc.vector.tensor_copy(
                            xt2[:, ACT_COLS:2 * C_IN], pt2[:, ACT_COLS:2 * C_IN]
                        )
                        xts.append(xt2)

                    # 16 accumulating matmuls for the whole 512-row load into
                    # one PSUM tile, then a single Sign op.
                    po4 = psum_o_pool.tile([128, CHUNKS_PER_LOAD * C_OUT], mybir.dt.float32)
                    for rc in range(CHUNKS_PER_LOAD):
                        xt2 = xts[rc // 2]
                        base = (rc % 2) * C_IN
                        for k in range(KC):
                            nc.tensor.matmul(
                                po4[:, rc * C_OUT:(rc + 1) * C_OUT],
                                xt2[:, base + k * 128:base + (k + 1) * 128],
                                w_sb[:, k * C_OUT:(k + 1) * C_OUT],
                                start=(k == 0),
                                stop=(k == KC - 1),
                            )
                    nc.scalar.sign(
                        o_sb[:, s * CHUNKS_PER_LOAD * C_OUT:(s + 1) * CHUNKS_PER_LOAD * C_OUT],
                        po4[:],
                    )
                nc.sync.dma_start(
                    out=y_out[g * rows_per_group:(g + 1) * rows_per_group, :].rearrange(
                        "(c p) n -> p c n", p=128
                    ),
                    in_=o_sb[:].rearrange("p (c n) -> p c n", n=C_OUT),
                )
    nc.finalize()
    return nc


def _get_nc(reps=1):
    if reps not in _NC:
        _NC[reps] = _build(reps)
    return _NC[reps]


def _prep_in_maps(x, W):
    x = np.asarray(x, dtype=np.float32)
    W = np.asarray(W, dtype=np.float32).reshape(C_IN, C_OUT)
    w_packed = np.ascontiguousarray(
        W.reshape(KC, 128, C_OUT).transpose(1, 0, 2).reshape(128, KC * C_OUT)
    )
    ident = np.eye(128, dtype=np.float32)
    shards = x.reshape(N_CORES, ROWS, C_IN)
    return [
        {"x": np.ascontiguousarray(shards[i]), "w": w_packed, "ident": ident}
        for i in range(N_CORES)
    ]


def _gather(results):
    out = np.stack([results[i]["y"] for i in range(N_CORES)], axis=0)
    return np.ascontiguousarray(out.reshape(B, H, W_DIM, C_OUT))


def kernel(x, W):
    nc = _get_nc()
    res = run_bass_kernel_spmd(nc, _prep_in_maps(x, W), core_ids=list(range(N_CORES)))
    return _gather(res.results)

